# revision 1
# baseline (speedup 1.0000x reference)
"""Trainium2 Bass kernel for single-head attention (B=4, S=2048, D=H=1024).

Sharding: 8 cores = 4 batches x 2 query-halves. Each core computes the
attention output for 1024 query rows of one batch; K/V are computed for the
batch's full sequence on both cores of the pair (no collectives needed).

Variants (PRECISE flag):
  fast:    all matmuls single-pass fp16 (PSUM fp32). ~2% absmax vs fp32 ref.
  precise: the score path (x->Q, x->K, Q.K^T) uses two-component fp16 hi/lo
           operands (3 matmul passes, ~22-bit effective operand precision);
           V / attn@V / out stay single fp16. ~0.05% absmax.

Per-core pipeline (v3):
  A1: QT[h,q] = Wq^T xq -> DRAM (tile-blocked, streamed back per q-tile)
  A2: KT[h,k] = Wk^T x (SBUF-resident) and V[k,h] = x^T Wv fused on the
      same streamed x chunks
  B:  per q-tile: S = QT^T KT -> rowmax (DVE negate) -> Exp(bias=-max,
      accum_out=den) -> En = E/den fp16 -> DRAM
  C:  per 512-query chunk: ET (DMA-transpose from DRAM; loads overlap B
      since the ETc tiles live in the early pool), yT = V^T ET (+bv),
      z^T = Wo^T yT (+bo) -> DRAM.  Host transposes z^T back to [B,S,D].
"""

import os
import sys

import numpy as np

for _p in ("/opt/trn_rl_repo",):
    if _p not in sys.path:
        sys.path.insert(0, _p)

import concourse.bass as bass
import concourse.mybir as mybir
import concourse.tile as tile
from concourse.bass_utils import run_bass_kernel_spmd


def _install_profile_shims():
    """This image's `antenv` lacks `axon_hooks`, which run_bass_kernel_spmd
    imports for trace=True under axon; libaxon_pjrt.so has the NTFF symbols.
    Register a stand-in module wired to the ctypes hook, and neuter the
    artifact upload (zero-egress container)."""
    import types

    try:
        import antenv.axon_hooks  # noqa: F401
    except ImportError:
        hook = None
        try:
            import trn_agent_boot.trn_boot as _tb

            hook = _tb._ntff_profile_via_ctypes("/opt/axon/libaxon_pjrt.so")
        except Exception:
            hook = None
        import antenv

        m = types.ModuleType("antenv.axon_hooks")
        m.get_axon_ntff_profile_hook = lambda: hook
        m.set_axon_ntff_profile_hook = lambda h: None
        sys.modules["antenv.axon_hooks"] = m
        antenv.axon_hooks = m

    import concourse.bass_utils as _bu

    _bu.upload_artifacts = lambda tmpdir: tmpdir


_install_profile_shims()

B, S, D, H = 4, 2048, 1024, 1024
P = 128
NQ = 1024  # query rows per core
D_T, H_T, S_T, Q_T = D // P, H // P, S // P, NQ // P
KC, QC, HC = S // 512, NQ // 512, H // 512

F32 = mybir.dt.float32
F16 = mybir.dt.float16
Ident = mybir.ActivationFunctionType.Identity

PRECISE = os.environ.get("ATTN_KERNEL_PRECISE", "1") == "1"


def _split_multi_waits(nc, max_waits=1):
    """This container's walrus rejects >1 sync wait on NO_STRUCT opcodes
    (Drain/NoOp). Move extra waits onto dedicated single-wait NoOps inserted
    right before the offending instruction on the same engine."""
    for f in nc.m.functions:
        for bb in f.blocks:
            insts = bb.instructions
            i = 0
            while i < len(insts):
                ins = insts[i]
                si = ins.sync_info
                if si is not None and si.on_wait and len(si.on_wait) > max_waits:
                    waits = list(si.on_wait)
                    si.on_wait = waits[:max_waits]
                    ins.sync_info = si
                    for j, w in enumerate(waits[max_waits:]):
                        nop = mybir.InstNoOp(
                            name=f"{ins.name}-waitsplit-{j}",
                            engine=ins.engine,
                            bass_nofuse=True,
                            sync_info=mybir.SyncInfo(on_wait=[w], on_update=[]),
                        )
                        insts.insert(i, nop)
                        i += 1
                i += 1
            bb.instructions = insts


def _build(precise=PRECISE, split_waits=True):
    nc = bass.Bass()

    def din(name, shape, dt=F16):
        return nc.declare_dram_parameter(name, shape, dt, isOutput=False)

    xTh = din("xTh", [D, S])
    xqh = din("xqh", [D, NQ])
    wqh = din("wqh", [D, H])
    wkh = din("wkh", [D, H])
    if precise:
        xTl, xql = din("xTl", [D, S]), din("xql", [D, NQ])
        wql, wkl = din("wql", [D, H]), din("wkl", [D, H])
    wv = din("wv", [D, H])
    wo = din("wo", [H, D])
    bq, bk = din("bq", [H], F32), din("bk", [H], F32)
    bv, bo = din("bv", [H], F32), din("bo", [D], F32)
    zT = nc.declare_dram_parameter("zT", [D, NQ], F32, isOutput=True)

    with tile.TileContext(nc) as tc:
        with (
            tc.tile_pool(name="pers", bufs=1) as pers,
            tc.tile_pool(name="dram", bufs=1, space="DRAM") as dramp,
            tc.tile_pool(name="ps", bufs=8, space="PSUM") as psp,
        ):
            bias_q = pers.tile([P, H_T], F32, tag="bq", name="bq")
            bias_k = pers.tile([P, H_T], F32, tag="bk", name="bk")
            bias_v = pers.tile([P, H_T], F32, tag="bv", name="bv")
            bias_o = pers.tile([P, D_T], F32, tag="bo", name="bo")
            nc.sync.dma_start(out=bias_q[:], in_=bq.rearrange("(t p) -> p t", p=P))
            nc.sync.dma_start(out=bias_k[:], in_=bk.rearrange("(t p) -> p t", p=P))
            nc.sync.dma_start(out=bias_v[:], in_=bv.rearrange("(t p) -> p t", p=P))
            nc.sync.dma_start(out=bias_o[:], in_=bo.rearrange("(t p) -> p t", p=P))
            EnD = dramp.tile([NQ, S], F16, tag="EnD", name="EnD")
            # QT staged to DRAM in [t][qt] 128x128 blocks (contiguous reads)
            QDh = dramp.tile([H_T, Q_T, P, P], F16, tag="QDh", name="QDh")
            if precise:
                QDl = dramp.tile([H_T, Q_T, P, P], F16, tag="QDl", name="QDl")

            def mm3(ps, wh, wl, xh, xl, sel, cs, first, last):
                """Accumulate (wh+wl)^T (xh+xl) ~ hi*hi + hi*lo + lo*hi."""
                nc.tensor.matmul(ps[:], wh[:, sel], xh[:, cs],
                                 start=first, stop=False)
                nc.tensor.matmul(ps[:], wh[:, sel], xl[:, cs],
                                 start=False, stop=False)
                nc.tensor.matmul(ps[:], wl[:, sel], xh[:, cs],
                                 start=False, stop=last)

            def split_hi_lo(ps, hi, lo, bias):
                """hi = f16(ps + bias); lo = f16((ps + bias) - hi)."""
                nc.scalar.activation(hi, ps, Ident, bias=bias)
                nc.vector.scalar_tensor_tensor(
                    out=lo, in0=ps, scalar=bias, in1=hi,
                    op0=mybir.AluOpType.add, op1=mybir.AluOpType.subtract,
                )

            # V and the transposed-attention chunks live across phases; this
            # pool opens first so their slots never overlap the KT pool (no
            # cross-phase WAR serialization, ETc loads can overlap phase B).
            with tc.tile_pool(name="pV", bufs=1) as pV:
                V = [pV.tile([P, H], F16, tag=f"v{s}", name=f"v{s}") for s in range(S_T)]
                ETc = [pV.tile([P, 512], F16, tag=f"et{s}", name=f"et{s}") for s in range(S_T)]

                with tc.tile_pool(name="score", bufs=1) as sc:
                    KTh = [sc.tile([P, S], F16, tag=f"kh{t}", name=f"kh{t}") for t in range(H_T)]
                    if precise:
                        KTl = [sc.tile([P, S], F16, tag=f"kl{t}", name=f"kl{t}") for t in range(H_T)]

                    # ---- A1: QT = Wq^T xq -> DRAM blocks ---------------
                    with tc.tile_pool(name="pA1", bufs=1) as pA1:
                        wqhs = [pA1.tile([P, H], F16, tag=f"wqh{d}", name=f"wqh{d}")
                                for d in range(D_T)]
                        if precise:
                            wqls = [pA1.tile([P, H], F16, tag=f"wql{d}", name=f"wql{d}")
                                    for d in range(D_T)]
                        for d in range(D_T):
                            r = slice(d * P, (d + 1) * P)
                            nc.sync.dma_start(out=wqhs[d][:], in_=wqh[r, :])
                            if precise:
                                nc.sync.dma_start(out=wqls[d][:], in_=wql[r, :])
                        for qc in range(QC):
                            cs = slice(qc * 512, (qc + 1) * 512)
                            xh_c, xl_c = [], []
                            for d in range(D_T):
                                r = slice(d * P, (d + 1) * P)
                                th = pA1.tile([P, 512], F16, tag=f"xqh{d}",
                                              name=f"xqh{d}", bufs=2)
                                nc.sync.dma_start(out=th[:], in_=xqh[r, cs])
                                xh_c.append(th)
                                if precise:
                                    tl = pA1.tile([P, 512], F16, tag=f"xql{d}",
                                                  name=f"xql{d}", bufs=2)
                                    nc.sync.dma_start(out=tl[:], in_=xql[r, cs])
                                    xl_c.append(tl)
                            fullc = slice(0, 512)
                            for t in range(H_T):
                                hs = slice(t * P, (t + 1) * P)
                                ps = psp.tile([P, 512], F32, tag="ps", name="ps")
                                for d in range(D_T):
                                    if precise:
                                        mm3(ps, wqhs[d], wqls[d], xh_c[d], xl_c[d],
                                            hs, fullc, d == 0, d == D_T - 1)
                                    else:
                                        nc.tensor.matmul(
                                            ps[:], wqhs[d][:, hs], xh_c[d][:, fullc],
                                            start=(d == 0), stop=(d == D_T - 1))
                                qh = pA1.tile([P, 512], F16, tag="qh",
                                              name="qh", bufs=2)
                                if precise:
                                    ql = pA1.tile([P, 512], F16, tag="ql",
                                                  name="ql", bufs=2)
                                    split_hi_lo(ps[:], qh[:], ql[:],
                                                bias_q[:, t : t + 1])
                                else:
                                    nc.scalar.activation(qh[:], ps[:], Ident,
                                                         bias=bias_q[:, t : t + 1])
                                for j in range(4):
                                    qt = qc * 4 + j
                                    js = slice(j * P, (j + 1) * P)
                                    nc.sync.dma_start(out=QDh[t, qt, :, :],
                                                      in_=qh[:, js])
                                    if precise:
                                        nc.sync.dma_start(out=QDl[t, qt, :, :],
                                                          in_=ql[:, js])

                    # ---- A2: KT (resident) + V fused on x chunks -------
                    with tc.tile_pool(name="pA2", bufs=1) as pA2:
                        wkhs = [pA2.tile([P, H], F16, tag=f"wkh{d}", name=f"wkh{d}")
                                for d in range(D_T)]
                        if precise:
                            wkls = [pA2.tile([P, H], F16, tag=f"wkl{d}", name=f"wkl{d}")
                                    for d in range(D_T)]
                        wvs = [pA2.tile([P, H], F16, tag=f"wv{d}", name=f"wv{d}")
                               for d in range(D_T)]
                        for d in range(D_T):
                            r = slice(d * P, (d + 1) * P)
                            nc.sync.dma_start(out=wkhs[d][:], in_=wkh[r, :])
                            if precise:
                                nc.sync.dma_start(out=wkls[d][:], in_=wkl[r, :])
                            nc.sync.dma_start(out=wvs[d][:], in_=wv[r, :])
                        for kc in range(KC):
                            cs = slice(kc * 512, (kc + 1) * 512)
                            xh_c, xl_c = [], []
                            for d in range(D_T):
                                r = slice(d * P, (d + 1) * P)
                                th = pA2.tile([P, 512], F16, tag=f"xkh{d}",
                                              name=f"xkh{d}", bufs=2)
                                nc.sync.dma_start(out=th[:], in_=xTh[r, cs])
                                xh_c.append(th)
                                if precise:
                                    tl = pA2.tile([P, 512], F16, tag=f"xkl{d}",
                                                  name=f"xkl{d}", bufs=1)
                                    nc.sync.dma_start(out=tl[:], in_=xTl[r, cs])
                                    xl_c.append(tl)
                            fullc = slice(0, 512)
                            for t in range(H_T):
                                hs = slice(t * P, (t + 1) * P)
                                ps = psp.tile([P, 512], F32, tag="ps", name="ps")
                                for d in range(D_T):
                                    if precise:
                                        mm3(ps, wkhs[d], wkls[d], xh_c[d], xl_c[d],
                                            hs, fullc, d == 0, d == D_T - 1)
                                    else:
                                        nc.tensor.matmul(
                                            ps[:], wkhs[d][:, hs], xh_c[d][:, fullc],
                                            start=(d == 0), stop=(d == D_T - 1))
                                if precise:
                                    split_hi_lo(ps[:], KTh[t][:, cs], KTl[t][:, cs],
                                                bias_k[:, t : t + 1])
                                else:
                                    nc.scalar.activation(KTh[t][:, cs], ps[:], Ident,
                                                         bias=bias_k[:, t : t + 1])
                            # V for this chunk's 4 k-tiles (x hi only)
                            for si in range(4):
                                s = kc * 4 + si
                                ksl = slice(si * P, (si + 1) * P)
                                for hc in range(HC):
                                    hcs = slice(hc * 512, (hc + 1) * 512)
                                    ps = psp.tile([P, 512], F32, tag="ps", name="ps")
                                    for d in range(D_T):
                                        nc.tensor.matmul(
                                            ps[:], xh_c[d][:, ksl], wvs[d][:, hcs],
                                            start=(d == 0), stop=(d == D_T - 1))
                                    nc.vector.tensor_copy(V[s][:, hcs], ps[:])

                    # ---- B: scores + softmax -> EnD --------------------
                    with tc.tile_pool(name="pB", bufs=2) as pB:
                        for qt in range(Q_T):
                            qs_full = slice(0, P)
                            qsh, qsl = [], []
                            for t in range(H_T):
                                sh = pB.tile([P, P], F16, tag=f"qsh{t}",
                                             name=f"qsh{t}", bufs=2)
                                nc.sync.dma_start(out=sh[:], in_=QDh[t, qt, :, :])
                                qsh.append(sh)
                                if precise:
                                    sl = pB.tile([P, P], F16, tag=f"qsl{t}",
                                                 name=f"qsl{t}", bufs=2)
                                    nc.sync.dma_start(out=sl[:], in_=QDl[t, qt, :, :])
                                    qsl.append(sl)
                            Ssb = pB.tile([P, S], F32, tag="Ssb", name="Ssb")
                            for kc in range(KC):
                                cs = slice(kc * 512, (kc + 1) * 512)
                                ps = psp.tile([P, 512], F32, tag="ps", name="ps")
                                for t in range(H_T):
                                    if precise:
                                        mm3(ps, qsh[t], qsl[t], KTh[t], KTl[t],
                                            qs_full, cs, t == 0, t == H_T - 1)
                                    else:
                                        nc.tensor.matmul(
                                            ps[:], qsh[t][:, qs_full], KTh[t][:, cs],
                                            start=(t == 0), stop=(t == H_T - 1))
                                nc.vector.tensor_copy(Ssb[:, cs], ps[:])
                            nmx = pB.tile([P, 1], F32, tag="nmx", name="nmx")
                            nc.vector.reduce_max(nmx[:], Ssb[:],
                                                 axis=mybir.AxisListType.X,
                                                 negate=True)
                            En = pB.tile([P, S], F16, tag="En", name="En")
                            den = pB.tile([P, 1], F32, tag="den", name="den")
                            nc.scalar.activation(
                                En[:], Ssb[:], mybir.ActivationFunctionType.Exp,
                                bias=nmx[:], accum_out=den[:])
                            rec = pB.tile([P, 1], F32, tag="rec", name="rec")
                            nc.vector.reciprocal(rec[:], den[:])
                            Enn = pB.tile([P, S], F16, tag="Enn", name="Enn")
                            nc.scalar.mul(Enn[:], En[:], rec[:])
                            nc.sync.dma_start(out=EnD[qt * P : (qt + 1) * P, :],
                                              in_=Enn[:])

                # ---- C: per q-chunk: ET load, yT, z -> DRAM ------------
                with tc.tile_pool(name="pC", bufs=1) as pC:
                    wos = [pC.tile([P, D], F16, tag=f"wo{t}", name=f"wo{t}")
                           for t in range(H_T)]
                    for t in range(H_T):
                        nc.sync.dma_start(out=wos[t][:],
                                          in_=wo[t * P : (t + 1) * P, :])
                    for qc in range(QC):
                        cs = slice(qc * 512, (qc + 1) * 512)
                        for s in range(S_T):
                            nc.sync.dma_start(
                                out=ETc[s][:],
                                in_=EnD[cs, s * P : (s + 1) * P],
                                transpose=True)
                        ycs = []
                        for t in range(H_T):
                            hs = slice(t * P, (t + 1) * P)
                            ps = psp.tile([P, 512], F32, tag="ps", name="ps")
                            for s in range(S_T):
                                nc.tensor.matmul(
                                    ps[:], V[s][:, hs], ETc[s][:],
                                    start=(s == 0), stop=(s == S_T - 1))
                            yc = pC.tile([P, 512], F16, tag=f"yc{t}",
                                         name=f"yc{t}", bufs=2)
                            nc.scalar.activation(yc[:], ps[:], Ident,
                                                 bias=bias_v[:, t : t + 1])
                            ycs.append(yc)
                        for td in range(D_T):
                            ds_ = slice(td * P, (td + 1) * P)
                            ps = psp.tile([P, 512], F32, tag="ps", name="ps")
                            for t in range(H_T):
                                nc.tensor.matmul(
                                    ps[:], wos[t][:, ds_], ycs[t][:],
                                    start=(t == 0), stop=(t == H_T - 1))
                            zsb = pC.tile([P, 512], F32, tag="zsb", name="zsb",
                                          bufs=2)
                            nc.scalar.activation(zsb[:], ps[:], Ident,
                                                 bias=bias_o[:, td : td + 1])
                            nc.sync.dma_start(out=zT[ds_, cs], in_=zsb[:])

    if split_waits:
        _split_multi_waits(nc)
    return nc


_NC = {}


def _get_nc(precise=PRECISE):
    if precise not in _NC:
        _NC[precise] = _build(precise=precise)
    return _NC[precise]


def _hi_lo(a):
    hi = a.astype(np.float16)
    lo = (a - hi.astype(np.float32)).astype(np.float16)
    return hi, lo


def _in_maps(x, Wq, bq, Wk, bk, Wv, bv, Wo, bo, precise=PRECISE):
    x = np.asarray(x, np.float32)
    xT = np.ascontiguousarray(np.transpose(x, (0, 2, 1)))  # [B, D, S] fp32
    com = {
        "wv": np.asarray(Wv, np.float16),
        "wo": np.asarray(Wo, np.float16),
        "bq": np.asarray(bq, np.float32),
        "bk": np.asarray(bk, np.float32),
        "bv": np.asarray(bv, np.float32),
        "bo": np.asarray(bo, np.float32),
    }
    if precise:
        com["wqh"], com["wql"] = _hi_lo(np.asarray(Wq, np.float32))
        com["wkh"], com["wkl"] = _hi_lo(np.asarray(Wk, np.float32))
        xTh, xTl = _hi_lo(xT)
    else:
        com["wqh"] = np.asarray(Wq, np.float16)
        com["wkh"] = np.asarray(Wk, np.float16)
        xTh = xT.astype(np.float16)
    maps = []
    for c in range(8):
        b, h = divmod(c, 2)
        qs = slice(h * NQ, (h + 1) * NQ)
        m = dict(com)
        m["xTh"] = xTh[b]
        m["xqh"] = np.ascontiguousarray(xTh[b][:, qs])
        if precise:
            m["xTl"] = xTl[b]
            m["xql"] = np.ascontiguousarray(xTl[b][:, qs])
        maps.append(m)
    return maps


def kernel(x, Wq, bq, Wk, bk, Wv, bv, Wo, bo, _trace=False, _precise=None):
    precise = PRECISE if _precise is None else _precise
    nc = _get_nc(precise)
    maps = _in_maps(x, Wq, bq, Wk, bk, Wv, bv, Wo, bo, precise=precise)
    res = run_bass_kernel_spmd(nc, maps, list(range(8)), trace=_trace)
    out = np.empty((B, S, D), np.float32)
    for c in range(8):
        b, h = divmod(c, 2)
        out[b, h * NQ : (h + 1) * NQ, :] = res.results[c]["zT"].T
    if _trace:
        kernel.last_exec_time_ns = res.exec_time_ns
        kernel.last_profile = res
    return out



# revision 2
# speedup vs baseline: 1.1055x; 1.1055x over previous
"""Trainium2 Bass kernel for single-head attention (B=4, S=2048, D=H=1024).

Sharding: 8 cores = 4 batches x 2 query-halves. Each core computes the
attention output for 1024 query rows of one batch; K/V are computed for the
batch's full sequence on both cores of the pair (no collectives needed).

All matmuls single-pass fp16 (PSUM fp32 accumulation); rel l2 vs the fp32
reference is ~3e-3 (softmax logits ~N(0, 32^2), fp16 q/k rounding perturbs
logits by ~0.02 which the peaked softmax amplifies to ~2% absmax).

v4 schedule (single fused projection pass, everything prefetched):
  x is fed per-core ROTATED so columns [0:1024] are that core's query half
  (the SPMD program is identical across cores; only data differs).
  QKV: stream x in 4 chunks of 512 seq-cols; per chunk compute KT (SBUF
       resident), V (SBUF resident), and - for the first 2 chunks - Q,
       staged to DRAM as [H_T, QC, 128, 512] blocks.
  B:   per 128-query tile: S = QT^T KT -> rowmax (negate) -> Exp(bias=-max,
       accum_out=den) -> En*recip -> fp16 rows to DRAM (EnDa: qtiles 0-3,
       EnDb: 4-7, split so C's transposed loads can start mid-B).
  C:   per 512-query chunk: ET (DMA-transpose of EnD half; chunk 0's tiles
       load while B runs qtiles 4-7), yT = V^T ET (+bv via softmax-sum=1),
       z^T = Wo^T yT (+bo) -> DRAM. Host transposes z^T back to [B,S,D].
"""

import sys

import numpy as np

for _p in ("/opt/trn_rl_repo",):
    if _p not in sys.path:
        sys.path.insert(0, _p)

import concourse.bass as bass
import concourse.mybir as mybir
import concourse.tile as tile
from concourse.bass_utils import run_bass_kernel_spmd


def _install_profile_shims():
    """This image's `antenv` lacks `axon_hooks`, which run_bass_kernel_spmd
    imports for trace=True under axon; libaxon_pjrt.so has the NTFF symbols.
    Register a stand-in module wired to the ctypes hook, and neuter the
    artifact upload (zero-egress container)."""
    import types

    try:
        import antenv.axon_hooks  # noqa: F401
    except ImportError:
        hook = None
        try:
            import trn_agent_boot.trn_boot as _tb

            hook = _tb._ntff_profile_via_ctypes("/opt/axon/libaxon_pjrt.so")
        except Exception:
            hook = None
        import antenv

        m = types.ModuleType("antenv.axon_hooks")
        m.get_axon_ntff_profile_hook = lambda: hook
        m.set_axon_ntff_profile_hook = lambda h: None
        sys.modules["antenv.axon_hooks"] = m
        antenv.axon_hooks = m

    import concourse.bass_utils as _bu

    _bu.upload_artifacts = lambda tmpdir: tmpdir


_install_profile_shims()

B, S, D, H = 4, 2048, 1024, 1024
P = 128
NQ = 1024  # query rows per core
D_T, H_T, S_T, Q_T = D // P, H // P, S // P, NQ // P
KC, QC, HC = S // 512, NQ // 512, H // 512

F32 = mybir.dt.float32
F16 = mybir.dt.float16
Ident = mybir.ActivationFunctionType.Identity


def _split_multi_waits(nc, max_waits=1):
    """This container's walrus rejects >1 sync wait on NO_STRUCT opcodes
    (Drain/NoOp). Move extra waits onto dedicated single-wait NoOps inserted
    right before the offending instruction on the same engine."""
    for f in nc.m.functions:
        for bb in f.blocks:
            insts = bb.instructions
            i = 0
            while i < len(insts):
                ins = insts[i]
                si = ins.sync_info
                if si is not None and si.on_wait and len(si.on_wait) > max_waits:
                    waits = list(si.on_wait)
                    si.on_wait = waits[:max_waits]
                    ins.sync_info = si
                    for j, w in enumerate(waits[max_waits:]):
                        nop = mybir.InstNoOp(
                            name=f"{ins.name}-waitsplit-{j}",
                            engine=ins.engine,
                            bass_nofuse=True,
                            sync_info=mybir.SyncInfo(on_wait=[w], on_update=[]),
                        )
                        insts.insert(i, nop)
                        i += 1
                i += 1
            bb.instructions = insts


def _build(split_waits=True):
    nc = bass.Bass()

    def din(name, shape, dt=F16):
        return nc.declare_dram_parameter(name, shape, dt, isOutput=False)

    xT = din("xT", [D, S])  # rotated: cols [0:NQ] are this core's queries
    wq = din("wq", [D, H])
    wk = din("wk", [D, H])
    wv = din("wv", [D, H])
    wo = din("wo", [H, D])
    bq, bk = din("bq", [H], F32), din("bk", [H], F32)
    bv, bo = din("bv", [H], F32), din("bo", [D], F32)
    zT = nc.declare_dram_parameter("zT", [D, NQ], F32, isOutput=True)

    with tile.TileContext(nc) as tc:
        with (
            tc.tile_pool(name="pers", bufs=1) as pers,
            tc.tile_pool(name="dram", bufs=1, space="DRAM") as dramp,
            tc.tile_pool(name="ps", bufs=8, space="PSUM") as psp,
        ):
            bias_q = pers.tile([P, H_T], F32, tag="bq", name="bq")
            bias_k = pers.tile([P, H_T], F32, tag="bk", name="bk")
            bias_v = pers.tile([P, H_T], F32, tag="bv", name="bv")
            bias_o = pers.tile([P, D_T], F32, tag="bo", name="bo")
            nc.sync.dma_start(out=bias_q[:], in_=bq.rearrange("(t p) -> p t", p=P))
            nc.sync.dma_start(out=bias_k[:], in_=bk.rearrange("(t p) -> p t", p=P))
            nc.sync.dma_start(out=bias_v[:], in_=bv.rearrange("(t p) -> p t", p=P))
            nc.sync.dma_start(out=bias_o[:], in_=bo.rearrange("(t p) -> p t", p=P))

            # Q staged to DRAM as [t][qc] 128x512 blocks (contiguous writes,
            # strided 128x128 reads in B).
            QD = dramp.tile([H_T, QC, P, 512], F16, tag="QD", name="QD")
            # En rows, split in two halves so C's transposed loads for query
            # chunk 0 depend only on qtiles 0-3.
            EnDa = dramp.tile([512, S], F16, tag="EnDa", name="EnDa")
            EnDb = dramp.tile([512, S], F16, tag="EnDb", name="EnDb")

            # Persistent SBUF: K^T, V, chunk-0 E^T tiles, and the K/V/O
            # weights. ~128 KiB/partition together with the biases.
            KT = [pers.tile([P, S], F16, tag=f"kt{t}", name=f"kt{t}") for t in range(H_T)]
            V = [pers.tile([P, H], F16, tag=f"v{s}", name=f"v{s}") for s in range(S_T)]
            ETa = [pers.tile([P, 512], F16, tag=f"eta{s}", name=f"eta{s}") for s in range(S_T)]
            wks = [pers.tile([P, H], F16, tag=f"wk{d}", name=f"wk{d}") for d in range(D_T)]
            wvs = [pers.tile([P, H], F16, tag=f"wv{d}", name=f"wv{d}") for d in range(D_T)]
            wos = [pers.tile([P, D], F16, tag=f"wo{t}", name=f"wo{t}") for t in range(H_T)]

            with tc.tile_pool(name="pqw", bufs=1) as pqw:
                wqs = [pqw.tile([P, H], F16, tag=f"wq{d}", name=f"wq{d}") for d in range(D_T)]
                # Load order = need order: wq (first matmuls), then wk, wv.
                # wo is emitted just before B so it queues behind the x loads.
                for d in range(D_T):
                    nc.sync.dma_start(out=wqs[d][:], in_=wq[d * P : (d + 1) * P, :])
                for d in range(D_T):
                    nc.sync.dma_start(out=wks[d][:], in_=wk[d * P : (d + 1) * P, :])
                for d in range(D_T):
                    nc.sync.dma_start(out=wvs[d][:], in_=wv[d * P : (d + 1) * P, :])

                # ---- fused QKV over 4 streamed x chunks -------------------
                with tc.tile_pool(name="px", bufs=1) as px:
                    for kc in range(KC):
                        cs = slice(kc * 512, (kc + 1) * 512)
                        xs = []
                        for d in range(D_T):
                            r = slice(d * P, (d + 1) * P)
                            t_ = px.tile([P, 512], F16, tag=f"x{d}", name=f"x{d}", bufs=2)
                            nc.sync.dma_start(out=t_[:], in_=xT[r, cs])
                            xs.append(t_)
                        fullc = slice(0, 512)
                        if kc < QC:  # Q for this core's query chunks
                            for t in range(H_T):
                                hs = slice(t * P, (t + 1) * P)
                                ps = psp.tile([P, 512], F32, tag="ps", name="ps")
                                for d in range(D_T):
                                    nc.tensor.matmul(
                                        ps[:], wqs[d][:, hs], xs[d][:, fullc],
                                        start=(d == 0), stop=(d == D_T - 1))
                                qh = px.tile([P, 512], F16, tag="qh", name="qh", bufs=2)
                                nc.scalar.activation(qh[:], ps[:], Ident,
                                                     bias=bias_q[:, t : t + 1])
                                nc.sync.dma_start(out=QD[t, kc, :, :], in_=qh[:])
                        for t in range(H_T):
                            hs = slice(t * P, (t + 1) * P)
                            ps = psp.tile([P, 512], F32, tag="ps", name="ps")
                            for d in range(D_T):
                                nc.tensor.matmul(
                                    ps[:], wks[d][:, hs], xs[d][:, fullc],
                                    start=(d == 0), stop=(d == D_T - 1))
                            nc.scalar.activation(KT[t][:, cs], ps[:], Ident,
                                                 bias=bias_k[:, t : t + 1])
                        for si in range(4):
                            s = kc * 4 + si
                            ksl = slice(si * P, (si + 1) * P)
                            for hc in range(HC):
                                hcs = slice(hc * 512, (hc + 1) * 512)
                                ps = psp.tile([P, 512], F32, tag="ps", name="ps")
                                for d in range(D_T):
                                    nc.tensor.matmul(
                                        ps[:], xs[d][:, ksl], wvs[d][:, hcs],
                                        start=(d == 0), stop=(d == D_T - 1))
                                # no +bv here: y = A(V + 1 bv^T) = AV + bv
                                # since softmax rows sum to 1; added in C.
                                nc.vector.tensor_copy(V[s][:, hcs], ps[:])

            # wo loads queue here: needed ~30us into C, issued during B.
            for t in range(H_T):
                nc.sync.dma_start(out=wos[t][:], in_=wo[t * P : (t + 1) * P, :])

            # ---- B: scores + softmax -> EnDa/EnDb ------------------------
            with tc.tile_pool(name="pe2", bufs=1) as pe2:
                ETb = [pe2.tile([P, 512], F16, tag=f"etb{s}", name=f"etb{s}")
                       for s in range(S_T)]
                with tc.tile_pool(name="pb", bufs=1) as pb:
                    for qt in range(Q_T):
                        qc, j = divmod(qt, 4)
                        js = slice(j * P, (j + 1) * P)
                        qsh = []
                        for t in range(H_T):
                            sh = pb.tile([P, P], F16, tag=f"qsh{t}",
                                         name=f"qsh{t}", bufs=2)
                            nc.sync.dma_start(out=sh[:], in_=QD[t, qc, :, js])
                            qsh.append(sh)
                        Ssb = pb.tile([P, S], F32, tag="Ssb", name="Ssb", bufs=2)
                        for kc in range(KC):
                            cs = slice(kc * 512, (kc + 1) * 512)
                            ps = psp.tile([P, 512], F32, tag="ps", name="ps")
                            for t in range(H_T):
                                nc.tensor.matmul(
                                    ps[:], qsh[t][:], KT[t][:, cs],
                                    start=(t == 0), stop=(t == H_T - 1))
                            nc.vector.tensor_copy(Ssb[:, cs], ps[:])
                        nmx = pb.tile([P, 1], F32, tag="nmx", name="nmx", bufs=2)
                        nc.vector.reduce_max(nmx[:], Ssb[:],
                                             axis=mybir.AxisListType.X,
                                             negate=True)
                        En = pb.tile([P, S], F16, tag="En", name="En")
                        den = pb.tile([P, 1], F32, tag="den", name="den", bufs=2)
                        nc.scalar.activation(
                            En[:], Ssb[:], mybir.ActivationFunctionType.Exp,
                            bias=nmx[:], accum_out=den[:])
                        rec = pb.tile([P, 1], F32, tag="rec", name="rec", bufs=2)
                        nc.vector.reciprocal(rec[:], den[:])
                        Enn = pb.tile([P, S], F16, tag="Enn", name="Enn", bufs=2)
                        nc.scalar.mul(Enn[:], En[:], rec[:])
                        EnD = EnDa if qt < 4 else EnDb
                        rs = slice((qt % 4) * P, (qt % 4 + 1) * P)
                        nc.sync.dma_start(out=EnD[rs, :], in_=Enn[:])
                        if qt == 3:
                            # E^T tiles for query chunk 0: overlap qtiles 4-7
                            for s in range(S_T):
                                nc.sync.dma_start(
                                    out=ETa[s][:],
                                    in_=EnDa[:, s * P : (s + 1) * P],
                                    transpose=True)

                # ---- C: per q-chunk: ET, yT = V^T ET, z = Wo^T yT --------
                with tc.tile_pool(name="pc", bufs=1) as pc:
                    for qc in range(QC):
                        cs = slice(qc * 512, (qc + 1) * 512)
                        if qc == 0:
                            # chunk-1 E^T loads: overlap chunk-0 compute
                            for s in range(S_T):
                                nc.sync.dma_start(
                                    out=ETb[s][:],
                                    in_=EnDb[:, s * P : (s + 1) * P],
                                    transpose=True)
                        ET = ETa if qc == 0 else ETb
                        ycs = []
                        for t in range(H_T):
                            hs = slice(t * P, (t + 1) * P)
                            ps = psp.tile([P, 512], F32, tag="ps", name="ps")
                            for s in range(S_T):
                                nc.tensor.matmul(
                                    ps[:], V[s][:, hs], ET[s][:],
                                    start=(s == 0), stop=(s == S_T - 1))
                            yc = pc.tile([P, 512], F16, tag=f"yc{t}",
                                         name=f"yc{t}", bufs=2)
                            nc.scalar.activation(yc[:], ps[:], Ident,
                                                 bias=bias_v[:, t : t + 1])
                            ycs.append(yc)
                        for td in range(D_T):
                            ds_ = slice(td * P, (td + 1) * P)
                            ps = psp.tile([P, 512], F32, tag="ps", name="ps")
                            for t in range(H_T):
                                nc.tensor.matmul(
                                    ps[:], wos[t][:, ds_], ycs[t][:],
                                    start=(t == 0), stop=(t == H_T - 1))
                            zsb = pc.tile([P, 512], F32, tag="zsb", name="zsb",
                                          bufs=2)
                            nc.scalar.activation(zsb[:], ps[:], Ident,
                                                 bias=bias_o[:, td : td + 1])
                            nc.sync.dma_start(out=zT[ds_, cs], in_=zsb[:])

    if split_waits:
        _split_multi_waits(nc)
    return nc


_NC = {}


def _get_nc():
    if "v4" not in _NC:
        _NC["v4"] = _build()
    return _NC["v4"]


def _in_maps(x, Wq, bq, Wk, bk, Wv, bv, Wo, bo):
    x = np.asarray(x, np.float32)
    xT = np.transpose(x, (0, 2, 1)).astype(np.float16)  # [B, D, S]
    com = {
        "wq": np.asarray(Wq, np.float16),
        "wk": np.asarray(Wk, np.float16),
        "wv": np.asarray(Wv, np.float16),
        "wo": np.asarray(Wo, np.float16),
        "bq": np.asarray(bq, np.float32),
        "bk": np.asarray(bk, np.float32),
        "bv": np.asarray(bv, np.float32),
        "bo": np.asarray(bo, np.float32),
    }
    maps = []
    for c in range(8):
        b, h = divmod(c, 2)
        m = dict(com)
        # rotate so this core's query half occupies columns [0:NQ]
        m["xT"] = np.ascontiguousarray(np.roll(xT[b], -h * NQ, axis=1))
        maps.append(m)
    return maps


def kernel(x, Wq, bq, Wk, bk, Wv, bv, Wo, bo, _trace=False, _precise=None):
    nc = _get_nc()
    maps = _in_maps(x, Wq, bq, Wk, bk, Wv, bv, Wo, bo)
    res = run_bass_kernel_spmd(nc, maps, list(range(8)), trace=_trace)
    out = np.empty((B, S, D), np.float32)
    for c in range(8):
        b, h = divmod(c, 2)
        out[b, h * NQ : (h + 1) * NQ, :] = res.results[c]["zT"].T
    if _trace:
        kernel.last_exec_time_ns = res.exec_time_ns
        kernel.last_profile = res
    return out


# revision 4
# speedup vs baseline: 1.1860x; 1.0728x over previous
"""Trainium2 Bass kernel for single-head attention (B=4, S=2048, D=H=1024).

Sharding: 8 cores = 4 batches x 2 query-halves. Each core computes the
attention output for 1024 query rows of one batch; K/V are computed for the
batch's full sequence on both cores of the pair (no collectives needed).

All matmuls single-pass fp16 (PSUM fp32 accumulation); rel l2 vs the fp32
reference is ~3e-3 (softmax logits ~N(0, 32^2), fp16 q/k rounding perturbs
logits by ~0.02 which the peaked softmax amplifies to ~2% absmax).

v5 schedule (single fused projection pass, on-chip transposes):
  x is fed per-core ROTATED so columns [0:1024] are that core's query half
  (the SPMD program is identical across cores; only data differs).
  DMA issue is split across the two HWDGE queues: weights/QD on ACT
  (nc.scalar), x/qsh/z on SP (nc.sync), so weight prefetch doesn't
  head-of-line-block the x stream at kernel start.
  QKV: stream x in 4 chunks of 512 seq-cols; per chunk compute KT (SBUF
       resident), V (SBUF resident), and - for the first 2 chunks - Q,
       staged to DRAM as [H_T, QC, 128, 512] blocks.
  B:   per 128-query tile: S = QT^T KT -> rowmax (negate) -> Exp(bias=-max,
       accum_out=den) -> En*recip -> 16 PE transposes (128x128, fp16 PSUM,
       packed 4 per bank) -> strided DVE copies into the per-chunk E^T
       supertile. No DRAM round trip; chunk-0 E^T is complete mid-B so C
       overlaps B's tail.
  C:   per 512-query chunk: yT = V^T ET (+bv via softmax-sum=1),
       z^T = Wo^T yT (+bo) -> DRAM. Host transposes z^T back to [B,S,D].
"""

import sys

import numpy as np

for _p in ("/opt/trn_rl_repo",):
    if _p not in sys.path:
        sys.path.insert(0, _p)

import concourse.bass as bass
import concourse.masks as masks
import concourse.mybir as mybir
import concourse.tile as tile
from concourse.bass_utils import run_bass_kernel_spmd


def _install_profile_shims():
    """This image's `antenv` lacks `axon_hooks`, which run_bass_kernel_spmd
    imports for trace=True under axon; libaxon_pjrt.so has the NTFF symbols.
    Register a stand-in module wired to the ctypes hook, and neuter the
    artifact upload (zero-egress container)."""
    import types

    try:
        import antenv.axon_hooks  # noqa: F401
    except ImportError:
        hook = None
        try:
            import trn_agent_boot.trn_boot as _tb

            hook = _tb._ntff_profile_via_ctypes("/opt/axon/libaxon_pjrt.so")
        except Exception:
            hook = None
        import antenv

        m = types.ModuleType("antenv.axon_hooks")
        m.get_axon_ntff_profile_hook = lambda: hook
        m.set_axon_ntff_profile_hook = lambda h: None
        sys.modules["antenv.axon_hooks"] = m
        antenv.axon_hooks = m

    import concourse.bass_utils as _bu

    _bu.upload_artifacts = lambda tmpdir: tmpdir


_install_profile_shims()

B, S, D, H = 4, 2048, 1024, 1024
P = 128
NQ = 1024  # query rows per core
D_T, H_T, S_T, Q_T = D // P, H // P, S // P, NQ // P
KC, QC, HC = S // 512, NQ // 512, H // 512

F32 = mybir.dt.float32
F16 = mybir.dt.float16
Ident = mybir.ActivationFunctionType.Identity


def _split_multi_waits(nc, max_waits=1):
    """This container's walrus rejects >1 sync wait on NO_STRUCT opcodes
    (Drain/NoOp). Move extra waits onto dedicated single-wait NoOps inserted
    right before the offending instruction on the same engine."""
    for f in nc.m.functions:
        for bb in f.blocks:
            insts = bb.instructions
            i = 0
            while i < len(insts):
                ins = insts[i]
                si = ins.sync_info
                if si is not None and si.on_wait and len(si.on_wait) > max_waits:
                    waits = list(si.on_wait)
                    si.on_wait = waits[:max_waits]
                    ins.sync_info = si
                    for j, w in enumerate(waits[max_waits:]):
                        nop = mybir.InstNoOp(
                            name=f"{ins.name}-waitsplit-{j}",
                            engine=ins.engine,
                            bass_nofuse=True,
                            sync_info=mybir.SyncInfo(on_wait=[w], on_update=[]),
                        )
                        insts.insert(i, nop)
                        i += 1
                i += 1
            bb.instructions = insts


def _build(split_waits=True):
    nc = bass.Bass()

    def din(name, shape, dt=F16):
        return nc.declare_dram_parameter(name, shape, dt, isOutput=False)

    xT = din("xT", [D, S])  # rotated: cols [0:NQ] are this core's queries
    wq = din("wq", [D, H])
    wk = din("wk", [D, H])
    wv = din("wv", [D, H])
    wo = din("wo", [H, D])
    bq, bk = din("bq", [H], F32), din("bk", [H], F32)
    bv, bo = din("bv", [H], F32), din("bo", [D], F32)
    zT = nc.declare_dram_parameter("zT", [D, NQ], F32, isOutput=True)

    with tile.TileContext(nc) as tc:
        with (
            tc.tile_pool(name="pers", bufs=1) as pers,
            tc.tile_pool(name="dram", bufs=1, space="DRAM") as dramp,
            tc.tile_pool(name="ps", bufs=6, space="PSUM") as psp,
            tc.tile_pool(name="pt", bufs=2, space="PSUM") as ptp,
        ):
            bias_q = pers.tile([P, H_T], F32, tag="bq", name="bq")
            bias_k = pers.tile([P, H_T], F32, tag="bk", name="bk")
            bias_v = pers.tile([P, H_T], F32, tag="bv", name="bv")
            bias_o = pers.tile([P, D_T], F32, tag="bo", name="bo")
            nc.scalar.dma_start(out=bias_q[:], in_=bq.rearrange("(t p) -> p t", p=P))
            nc.scalar.dma_start(out=bias_k[:], in_=bk.rearrange("(t p) -> p t", p=P))
            nc.scalar.dma_start(out=bias_v[:], in_=bv.rearrange("(t p) -> p t", p=P))
            nc.scalar.dma_start(out=bias_o[:], in_=bo.rearrange("(t p) -> p t", p=P))
            ident = pers.tile([P, P], F16, tag="ident", name="ident")
            masks.make_identity(nc, ident[:])

            # Q staged to DRAM as [t][qc] 128x512 blocks (contiguous writes,
            # strided 128x128 reads in B).
            QD = dramp.tile([H_T, QC, P, 512], F16, tag="QD", name="QD")

            # Persistent SBUF: K^T, V, chunk-0 E^T supertile, and the K/V/O
            # weights. ~128 KiB/partition together with the biases.
            KT = [pers.tile([P, S], F16, tag=f"kt{t}", name=f"kt{t}") for t in range(H_T)]
            V = [pers.tile([P, H], F16, tag=f"v{s}", name=f"v{s}") for s in range(S_T)]
            # E^T for query chunk 0: [k, q] with k tile s at cols s*512..
            ETa = pers.tile([P, S_T * 512], F16, tag="eta", name="eta")
            wks = [pers.tile([P, H], F16, tag=f"wk{d}", name=f"wk{d}") for d in range(D_T)]
            wvs = [pers.tile([P, H], F16, tag=f"wv{d}", name=f"wv{d}") for d in range(D_T)]
            wos = [pers.tile([P, D], F16, tag=f"wo{t}", name=f"wo{t}") for t in range(H_T)]

            with tc.tile_pool(name="pqw", bufs=1) as pqw:
                wqs = [pqw.tile([P, H], F16, tag=f"wq{d}", name=f"wq{d}") for d in range(D_T)]
                # Weight prefetch on the ACT HWDGE queue, in need order;
                # x stream below is alone on the SP queue.
                for d in range(D_T):
                    nc.scalar.dma_start(out=wqs[d][:], in_=wq[d * P : (d + 1) * P, :])
                for d in range(D_T):
                    nc.scalar.dma_start(out=wks[d][:], in_=wk[d * P : (d + 1) * P, :])
                for d in range(D_T):
                    nc.scalar.dma_start(out=wvs[d][:], in_=wv[d * P : (d + 1) * P, :])

                # ---- fused QKV over 4 streamed x chunks -------------------
                with tc.tile_pool(name="px", bufs=1) as px:
                    for kc in range(KC):
                        cs = slice(kc * 512, (kc + 1) * 512)
                        xs = []
                        for d in range(D_T):
                            r = slice(d * P, (d + 1) * P)
                            t_ = px.tile([P, 512], F16, tag=f"x{d}", name=f"x{d}", bufs=2)
                            nc.sync.dma_start(out=t_[:], in_=xT[r, cs])
                            xs.append(t_)
                        fullc = slice(0, 512)
                        if kc < QC:  # Q for this core's query chunks
                            for t in range(H_T):
                                hs = slice(t * P, (t + 1) * P)
                                ps = psp.tile([P, 512], F32, tag="ps", name="ps")
                                for d in range(D_T):
                                    nc.tensor.matmul(
                                        ps[:], wqs[d][:, hs], xs[d][:, fullc],
                                        start=(d == 0), stop=(d == D_T - 1))
                                qh = px.tile([P, 512], F16, tag="qh", name="qh", bufs=2)
                                nc.scalar.activation(qh[:], ps[:], Ident,
                                                     bias=bias_q[:, t : t + 1])
                                nc.scalar.dma_start(out=QD[t, kc, :, :], in_=qh[:])
                        for t in range(H_T):
                            hs = slice(t * P, (t + 1) * P)
                            ps = psp.tile([P, 512], F32, tag="ps", name="ps")
                            for d in range(D_T):
                                nc.tensor.matmul(
                                    ps[:], wks[d][:, hs], xs[d][:, fullc],
                                    start=(d == 0), stop=(d == D_T - 1))
                            nc.scalar.activation(KT[t][:, cs], ps[:], Ident,
                                                 bias=bias_k[:, t : t + 1])
                        for si in range(4):
                            s = kc * 4 + si
                            ksl = slice(si * P, (si + 1) * P)
                            for hc in range(HC):
                                hcs = slice(hc * 512, (hc + 1) * 512)
                                ps = psp.tile([P, 512], F32, tag="ps", name="ps")
                                for d in range(D_T):
                                    nc.tensor.matmul(
                                        ps[:], xs[d][:, ksl], wvs[d][:, hcs],
                                        start=(d == 0), stop=(d == D_T - 1))
                                # no +bv here: y = A(V + 1 bv^T) = AV + bv
                                # since softmax rows sum to 1; added in C.
                                nc.vector.tensor_copy(V[s][:, hcs], ps[:])

            # wo prefetch: needed ~30us into C, issued during B on ACT.
            for t in range(H_T):
                nc.scalar.dma_start(out=wos[t][:], in_=wo[t * P : (t + 1) * P, :])

            # ---- B: scores + softmax + on-chip transpose -----------------
            with tc.tile_pool(name="pe2", bufs=1) as pe2:
                ETb = pe2.tile([P, S_T * 512], F16, tag="etb", name="etb")
                with tc.tile_pool(name="pb", bufs=1) as pb:
                    def qsh_load(qt):
                        qc, j = divmod(qt, 4)
                        js = slice(j * P, (j + 1) * P)
                        tiles = []
                        for t in range(H_T):
                            sh = pb.tile([P, P], F16, tag=f"qsh{t}",
                                         name=f"qsh{t}", bufs=2)
                            nc.sync.dma_start(out=sh[:], in_=QD[t, qc, :, js])
                            tiles.append(sh)
                        return tiles

                    qsh_pre = {0: qsh_load(0), 1: qsh_load(1)}
                    for qt in range(Q_T):
                        qsh = qsh_pre.pop(qt)
                        if qt + 2 < Q_T:
                            qsh_pre[qt + 2] = qsh_load(qt + 2)
                        Ssb = pb.tile([P, S], F32, tag="Ssb", name="Ssb", bufs=2)
                        for kc in range(KC):
                            cs = slice(kc * 512, (kc + 1) * 512)
                            ps = psp.tile([P, 512], F32, tag="ps", name="ps")
                            for t in range(H_T):
                                nc.tensor.matmul(
                                    ps[:], qsh[t][:], KT[t][:, cs],
                                    start=(t == 0), stop=(t == H_T - 1))
                            nc.vector.tensor_copy(Ssb[:, cs], ps[:])
                        nmx = pb.tile([P, 1], F32, tag="nmx", name="nmx", bufs=2)
                        nc.vector.reduce_max(nmx[:], Ssb[:],
                                             axis=mybir.AxisListType.X,
                                             negate=True)
                        En = pb.tile([P, S], F16, tag="En", name="En")
                        den = pb.tile([P, 1], F32, tag="den", name="den", bufs=2)
                        nc.scalar.activation(
                            En[:], Ssb[:], mybir.ActivationFunctionType.Exp,
                            bias=nmx[:], accum_out=den[:])
                        rec = pb.tile([P, 1], F32, tag="rec", name="rec", bufs=2)
                        nc.vector.reciprocal(rec[:], den[:])
                        Enn = pb.tile([P, S], F16, tag="Enn", name="Enn", bufs=2)
                        nc.scalar.mul(Enn[:], En[:], rec[:])
                        # On-chip transpose: Enn [q, k] -> ET [k, q], 16
                        # 128x128 PE transposes packed 4 per fp16 PSUM bank,
                        # then one strided DVE copy per pack of 4.
                        ET = ETa if qt < 4 else ETb
                        j = qt % 4
                        for a in range(4):
                            pst = ptp.tile([P, 512], F16, tag="pst", name="pst")
                            for i in range(4):
                                s = 4 * a + i
                                nc.tensor.transpose(
                                    pst[:, i * P : (i + 1) * P],
                                    Enn[:, s * P : (s + 1) * P],
                                    ident[:])
                            src = pst[:].rearrange("p (i c) -> p i c", c=P)
                            dst = ET[:].rearrange("p (s c) -> p s c", c=512)[
                                :, 4 * a : 4 * a + 4, j * P : (j + 1) * P]
                            nc.vector.tensor_copy(dst, src)

                # ---- C: per q-chunk: yT = V^T ET, z = Wo^T yT ------------
                with tc.tile_pool(name="pc", bufs=1) as pc:
                    for qc in range(QC):
                        cs = slice(qc * 512, (qc + 1) * 512)
                        ET = ETa if qc == 0 else ETb
                        ycs = []
                        for t in range(H_T):
                            hs = slice(t * P, (t + 1) * P)
                            ps = psp.tile([P, 512], F32, tag="ps", name="ps")
                            for s in range(S_T):
                                nc.tensor.matmul(
                                    ps[:], V[s][:, hs],
                                    ET[:, s * 512 : (s + 1) * 512],
                                    start=(s == 0), stop=(s == S_T - 1))
                            yc = pc.tile([P, 512], F16, tag=f"yc{t}",
                                         name=f"yc{t}", bufs=2)
                            nc.scalar.activation(yc[:], ps[:], Ident,
                                                 bias=bias_v[:, t : t + 1])
                            ycs.append(yc)
                        for td in range(D_T):
                            ds_ = slice(td * P, (td + 1) * P)
                            ps = psp.tile([P, 512], F32, tag="ps", name="ps")
                            for t in range(H_T):
                                nc.tensor.matmul(
                                    ps[:], wos[t][:, ds_], ycs[t][:],
                                    start=(t == 0), stop=(t == H_T - 1))
                            zsb = pc.tile([P, 512], F32, tag="zsb", name="zsb",
                                          bufs=2)
                            nc.scalar.activation(zsb[:], ps[:], Ident,
                                                 bias=bias_o[:, td : td + 1])
                            nc.sync.dma_start(out=zT[ds_, cs], in_=zsb[:])

    if split_waits:
        _split_multi_waits(nc)
    return nc


_NC = {}


def _get_nc():
    if "v5" not in _NC:
        _NC["v5"] = _build()
    return _NC["v5"]


def _in_maps(x, Wq, bq, Wk, bk, Wv, bv, Wo, bo):
    x = np.asarray(x, np.float32)
    xT = np.transpose(x, (0, 2, 1)).astype(np.float16)  # [B, D, S]
    com = {
        "wq": np.asarray(Wq, np.float16),
        "wk": np.asarray(Wk, np.float16),
        "wv": np.asarray(Wv, np.float16),
        "wo": np.asarray(Wo, np.float16),
        "bq": np.asarray(bq, np.float32),
        "bk": np.asarray(bk, np.float32),
        "bv": np.asarray(bv, np.float32),
        "bo": np.asarray(bo, np.float32),
    }
    maps = []
    for c in range(8):
        b, h = divmod(c, 2)
        m = dict(com)
        # rotate so this core's query half occupies columns [0:NQ]
        m["xT"] = np.ascontiguousarray(np.roll(xT[b], -h * NQ, axis=1))
        maps.append(m)
    return maps


def kernel(x, Wq, bq, Wk, bk, Wv, bv, Wo, bo, _trace=False, _precise=None):
    nc = _get_nc()
    maps = _in_maps(x, Wq, bq, Wk, bk, Wv, bv, Wo, bo)
    res = run_bass_kernel_spmd(nc, maps, list(range(8)), trace=_trace)
    out = np.empty((B, S, D), np.float32)
    for c in range(8):
        b, h = divmod(c, 2)
        out[b, h * NQ : (h + 1) * NQ, :] = res.results[c]["zT"].T
    if _trace:
        kernel.last_exec_time_ns = res.exec_time_ns
        kernel.last_profile = res
    return out


# revision 11
# speedup vs baseline: 1.2132x; 1.0230x over previous
"""Trainium2 Bass kernel for single-head attention (B=4, S=2048, D=H=1024).

Sharding: 8 cores = 4 batches x 2 query-halves. Each core computes the
attention output for 1024 query rows of one batch; K/V are computed for the
batch's full sequence on both cores of the pair (no collectives needed).

All matmuls single-pass fp16 (PSUM fp32 accumulation); rel l2 vs the fp32
reference is ~3e-3 (softmax logits ~N(0, 32^2), fp16 q/k rounding perturbs
logits by ~0.02 which the peaked softmax amplifies to ~2% absmax).

v5 schedule (single fused projection pass, on-chip transposes):
  x is fed per-core ROTATED so columns [0:1024] are that core's query half
  (the SPMD program is identical across cores; only data differs).
  DMA issue is split across the two HWDGE queues: weights/QD on ACT
  (nc.scalar), x/qsh/z on SP (nc.sync), so weight prefetch doesn't
  head-of-line-block the x stream at kernel start.
  QKV: stream x in 4 chunks of 512 seq-cols; per chunk compute KT (SBUF
       resident), V (SBUF resident), and - for the first 2 chunks - Q,
       staged to DRAM as [H_T, QC, 128, 512] blocks.
  B:   per 128-query tile: S = QT^T KT -> rowmax (negate) -> Exp(bias=-max,
       accum_out=den) -> En*recip -> 16 PE transposes (128x128, fp16 PSUM,
       packed 4 per bank) -> strided DVE copies into the per-chunk E^T
       supertile. No DRAM round trip; chunk-0 E^T is complete mid-B so C
       overlaps B's tail.
  C:   per 512-query chunk: yT = V^T ET (+bv via softmax-sum=1),
       z^T = Wo^T yT (+bo) -> DRAM. Host transposes z^T back to [B,S,D].
"""

import sys

import numpy as np

for _p in ("/opt/trn_rl_repo",):
    if _p not in sys.path:
        sys.path.insert(0, _p)

import concourse.bass as bass
import concourse.masks as masks
import concourse.mybir as mybir
import concourse.tile as tile
from concourse.bass_utils import run_bass_kernel_spmd


def _install_profile_shims():
    """This image's `antenv` lacks `axon_hooks`, which run_bass_kernel_spmd
    imports for trace=True under axon; libaxon_pjrt.so has the NTFF symbols.
    Register a stand-in module wired to the ctypes hook, and neuter the
    artifact upload (zero-egress container)."""
    import types

    try:
        import antenv.axon_hooks  # noqa: F401
    except ImportError:
        hook = None
        try:
            import trn_agent_boot.trn_boot as _tb

            hook = _tb._ntff_profile_via_ctypes("/opt/axon/libaxon_pjrt.so")
        except Exception:
            hook = None
        import antenv

        m = types.ModuleType("antenv.axon_hooks")
        m.get_axon_ntff_profile_hook = lambda: hook
        m.set_axon_ntff_profile_hook = lambda h: None
        sys.modules["antenv.axon_hooks"] = m
        antenv.axon_hooks = m

    import concourse.bass_utils as _bu

    _bu.upload_artifacts = lambda tmpdir: tmpdir


_install_profile_shims()

B, S, D, H = 4, 2048, 1024, 1024
P = 128
NQ = 1024  # query rows per core
D_T, H_T, S_T, Q_T = D // P, H // P, S // P, NQ // P
KC, QC, HC = S // 512, NQ // 512, H // 512

F32 = mybir.dt.float32
F16 = mybir.dt.float16
Ident = mybir.ActivationFunctionType.Identity


def _split_multi_waits(nc, max_waits=1):
    """This container's walrus rejects >1 sync wait on NO_STRUCT opcodes
    (Drain/NoOp). Move extra waits onto dedicated single-wait NoOps inserted
    right before the offending instruction on the same engine."""
    for f in nc.m.functions:
        for bb in f.blocks:
            insts = bb.instructions
            i = 0
            while i < len(insts):
                ins = insts[i]
                si = ins.sync_info
                if si is not None and si.on_wait and len(si.on_wait) > max_waits:
                    waits = list(si.on_wait)
                    si.on_wait = waits[:max_waits]
                    ins.sync_info = si
                    for j, w in enumerate(waits[max_waits:]):
                        nop = mybir.InstNoOp(
                            name=f"{ins.name}-waitsplit-{j}",
                            engine=ins.engine,
                            bass_nofuse=True,
                            sync_info=mybir.SyncInfo(on_wait=[w], on_update=[]),
                        )
                        insts.insert(i, nop)
                        i += 1
                i += 1
            bb.instructions = insts


def _build(split_waits=True):
    nc = bass.Bass()

    def din(name, shape, dt=F16):
        return nc.declare_dram_parameter(name, shape, dt, isOutput=False)

    xT = din("xT", [D, S])  # rotated: cols [0:NQ] are this core's queries
    wq = din("wq", [D, H])
    wk = din("wk", [D, H])
    wv = din("wv", [D, H])
    wo = din("wo", [H, D])
    bq, bk = din("bq", [H], F32), din("bk", [H], F32)
    bv, bo = din("bv", [H], F32), din("bo", [D], F32)
    zT = nc.declare_dram_parameter("zT", [D, NQ], F32, isOutput=True)

    with tile.TileContext(nc) as tc:
        with (
            tc.tile_pool(name="pers", bufs=1) as pers,
            tc.tile_pool(name="dram", bufs=1, space="DRAM") as dramp,
            tc.tile_pool(name="ps", bufs=6, space="PSUM") as psp,
            tc.tile_pool(name="pt", bufs=2, space="PSUM") as ptp,
        ):
            bias_q = pers.tile([P, H_T], F32, tag="bq", name="bq")
            bias_k = pers.tile([P, H_T], F32, tag="bk", name="bk")
            bias_v = pers.tile([P, H_T], F32, tag="bv", name="bv")
            bias_o = pers.tile([P, D_T], F32, tag="bo", name="bo")
            ident = pers.tile([P, P], F16, tag="ident", name="ident")
            masks.make_identity(nc, ident[:])

            # Q staged to DRAM as [t][qc] 128x512 blocks (contiguous writes,
            # strided 128x128 reads in B).
            QD = dramp.tile([H_T, QC, P, 512], F16, tag="QD", name="QD")

            # Persistent SBUF: K^T, V, chunk-0 E^T supertile, and the K/V/O
            # weights. ~128 KiB/partition together with the biases.
            KT = [pers.tile([P, S], F16, tag=f"kt{t}", name=f"kt{t}") for t in range(H_T)]
            V = [pers.tile([P, H], F16, tag=f"v{s}", name=f"v{s}") for s in range(S_T)]
            # E^T for query chunk 0: [k, q] with k tile s at cols s*512..
            ETa = pers.tile([P, S_T * 512], F16, tag="eta", name="eta")
            wks = [pers.tile([P, H], F16, tag=f"wk{d}", name=f"wk{d}") for d in range(D_T)]
            wvs = [pers.tile([P, H], F16, tag=f"wv{d}", name=f"wv{d}") for d in range(D_T)]
            wos = [pers.tile([P, D], F16, tag=f"wo{t}", name=f"wo{t}") for t in range(H_T)]
            # qsh tags live at top level so B's first q-tiles can prefetch
            # during the QKV phase.
            def qsh_load(qt):
                qc, j = divmod(qt, 4)
                js = slice(j * P, (j + 1) * P)
                tiles = []
                for t in range(H_T):
                    sh = pers.tile([P, P], F16, tag=f"qsh{t}",
                                   name=f"qsh{t}", bufs=2)
                    nc.sync.dma_start(out=sh[:], in_=QD[t, qc, :, js])
                    tiles.append(sh)
                return tiles

            qsh_pre = {}

            with tc.tile_pool(name="pqw", bufs=1) as pqw:
                wqs = [pqw.tile([P, H], F16, tag=f"wq{d}", name=f"wq{d}") for d in range(D_T)]
                # Weight prefetch on the ACT HWDGE queue, in need order;
                # x stream below is alone on the SP queue. wv, biases, wo
                # defer until after the first chunk's deps.
                for d in range(D_T):
                    nc.scalar.dma_start(out=wqs[d][:], in_=wq[d * P : (d + 1) * P, :])
                for d in range(D_T):
                    nc.scalar.dma_start(out=wks[d][:], in_=wk[d * P : (d + 1) * P, :])

                # ---- fused QKV over 4 streamed x chunks -------------------
                with tc.tile_pool(name="px", bufs=1) as px:
                    for kc in range(KC):
                        cs = slice(kc * 512, (kc + 1) * 512)
                        xs = []
                        for d in range(D_T):
                            r = slice(d * P, (d + 1) * P)
                            t_ = px.tile([P, 512], F16, tag=f"x{d}", name=f"x{d}", bufs=2)
                            nc.sync.dma_start(out=t_[:], in_=xT[r, cs])
                            xs.append(t_)
                        if kc == 0:
                            nc.scalar.dma_start(
                                out=bias_q[:], in_=bq.rearrange("(t p) -> p t", p=P))
                            nc.scalar.dma_start(
                                out=bias_k[:], in_=bk.rearrange("(t p) -> p t", p=P))
                            nc.scalar.dma_start(
                                out=bias_v[:], in_=bv.rearrange("(t p) -> p t", p=P))
                            nc.scalar.dma_start(
                                out=bias_o[:], in_=bo.rearrange("(t p) -> p t", p=P))
                        fullc = slice(0, 512)
                        if kc < QC:  # Q for this core's query chunks
                            for t in range(H_T):
                                hs = slice(t * P, (t + 1) * P)
                                ps = psp.tile([P, 512], F32, tag="ps", name="ps")
                                for d in range(D_T):
                                    nc.tensor.matmul(
                                        ps[:], wqs[d][:, hs], xs[d][:, fullc],
                                        start=(d == 0), stop=(d == D_T - 1))
                                qh = px.tile([P, 512], F16, tag="qh", name="qh", bufs=2)
                                nc.scalar.activation(qh[:], ps[:], Ident,
                                                     bias=bias_q[:, t : t + 1])
                                nc.scalar.dma_start(out=QD[t, kc, :, :], in_=qh[:])
                        for t in range(H_T):
                            hs = slice(t * P, (t + 1) * P)
                            ps = psp.tile([P, 512], F32, tag="ps", name="ps")
                            for d in range(D_T):
                                nc.tensor.matmul(
                                    ps[:], wks[d][:, hs], xs[d][:, fullc],
                                    start=(d == 0), stop=(d == D_T - 1))
                            nc.scalar.activation(KT[t][:, cs], ps[:], Ident,
                                                 bias=bias_k[:, t : t + 1])
                        if kc == 0:
                            for d in range(D_T):
                                nc.scalar.dma_start(
                                    out=wvs[d][:], in_=wv[d * P : (d + 1) * P, :])
                        for si in range(4):
                            s = kc * 4 + si
                            ksl = slice(si * P, (si + 1) * P)
                            for hc in range(HC):
                                hcs = slice(hc * 512, (hc + 1) * 512)
                                ps = psp.tile([P, 512], F32, tag="ps", name="ps")
                                for d in range(D_T):
                                    nc.tensor.matmul(
                                        ps[:], xs[d][:, ksl], wvs[d][:, hcs],
                                        start=(d == 0), stop=(d == D_T - 1))
                                # no +bv here: y = A(V + 1 bv^T) = AV + bv
                                # since softmax rows sum to 1; added in C.
                                nc.vector.tensor_copy(V[s][:, hcs], ps[:])
                        if kc == 1:
                            qsh_pre[0] = qsh_load(0)
                            qsh_pre[1] = qsh_load(1)

            # wo prefetch: needed ~30us into C, issued during B on ACT.
            for t in range(H_T):
                nc.scalar.dma_start(out=wos[t][:], in_=wo[t * P : (t + 1) * P, :])

            # ---- B: scores + softmax + on-chip transpose -----------------
            with tc.tile_pool(name="pe2", bufs=1) as pe2:
                ETb = pe2.tile([P, S_T * 512], F16, tag="etb", name="etb")
                with tc.tile_pool(name="pb", bufs=1) as pb:
                    for qt in range(Q_T):
                        qsh = qsh_pre.pop(qt)
                        if qt + 2 < Q_T:
                            qsh_pre[qt + 2] = qsh_load(qt + 2)
                        Ssb = pb.tile([P, S], F32, tag="Ssb", name="Ssb", bufs=2)
                        for kc in range(KC):
                            cs = slice(kc * 512, (kc + 1) * 512)
                            ps = psp.tile([P, 512], F32, tag="ps", name="ps")
                            for t in range(H_T):
                                nc.tensor.matmul(
                                    ps[:], qsh[t][:], KT[t][:, cs],
                                    start=(t == 0), stop=(t == H_T - 1))
                            nc.vector.tensor_copy(Ssb[:, cs], ps[:])
                        nmx = pb.tile([P, 1], F32, tag="nmx", name="nmx", bufs=2)
                        nc.vector.reduce_max(nmx[:], Ssb[:],
                                             axis=mybir.AxisListType.X,
                                             negate=True)
                        En = pb.tile([P, S], F16, tag="En", name="En")
                        den = pb.tile([P, 1], F32, tag="den", name="den", bufs=2)
                        nc.scalar.activation(
                            En[:], Ssb[:], mybir.ActivationFunctionType.Exp,
                            bias=nmx[:], accum_out=den[:])
                        rec = pb.tile([P, 1], F32, tag="rec", name="rec", bufs=2)
                        nc.vector.reciprocal(rec[:], den[:])
                        Enn = pb.tile([P, S], F16, tag="Enn", name="Enn", bufs=2)
                        nc.scalar.mul(Enn[:], En[:], rec[:])
                        # On-chip transpose: Enn [q, k] -> ET [k, q], 16
                        # 128x128 PE transposes packed 4 per fp16 PSUM bank,
                        # then one strided DVE copy per pack of 4.
                        ET = ETa if qt < 4 else ETb
                        j = qt % 4
                        for a in range(4):
                            pst = ptp.tile([P, 512], F16, tag="pst", name="pst")
                            for i in range(4):
                                s = 4 * a + i
                                nc.tensor.transpose(
                                    pst[:, i * P : (i + 1) * P],
                                    Enn[:, s * P : (s + 1) * P],
                                    ident[:])
                            src = pst[:].rearrange("p (i c) -> p i c", c=P)
                            dst = ET[:].rearrange("p (s c) -> p s c", c=512)[
                                :, 4 * a : 4 * a + 4, j * P : (j + 1) * P]
                            nc.vector.tensor_copy(dst, src)

                # ---- C: per q-chunk: yT = V^T ET, z = Wo^T yT ------------
                with tc.tile_pool(name="pc", bufs=1) as pc:
                    for qc in range(QC):
                        cs = slice(qc * 512, (qc + 1) * 512)
                        ET = ETa if qc == 0 else ETb
                        ycs = []
                        for t in range(H_T):
                            hs = slice(t * P, (t + 1) * P)
                            ps = psp.tile([P, 512], F32, tag="ps", name="ps")
                            for s in range(S_T):
                                nc.tensor.matmul(
                                    ps[:], V[s][:, hs],
                                    ET[:, s * 512 : (s + 1) * 512],
                                    start=(s == 0), stop=(s == S_T - 1))
                            yc = pc.tile([P, 512], F16, tag=f"yc{t}",
                                         name=f"yc{t}", bufs=2)
                            nc.scalar.activation(yc[:], ps[:], Ident,
                                                 bias=bias_v[:, t : t + 1])
                            ycs.append(yc)
                        for td in range(D_T):
                            ds_ = slice(td * P, (td + 1) * P)
                            ps = psp.tile([P, 512], F32, tag="ps", name="ps")
                            for t in range(H_T):
                                nc.tensor.matmul(
                                    ps[:], wos[t][:, ds_], ycs[t][:],
                                    start=(t == 0), stop=(t == H_T - 1))
                            zsb = pc.tile([P, 512], F32, tag="zsb", name="zsb",
                                          bufs=2)
                            nc.scalar.activation(zsb[:], ps[:], Ident,
                                                 bias=bias_o[:, td : td + 1])
                            nc.sync.dma_start(out=zT[ds_, cs], in_=zsb[:])

    if split_waits:
        _split_multi_waits(nc)
    return nc


_NC = {}


def _get_nc():
    if "v5" not in _NC:
        _NC["v5"] = _build()
    return _NC["v5"]


def _in_maps(x, Wq, bq, Wk, bk, Wv, bv, Wo, bo):
    x = np.asarray(x, np.float32)
    xT = np.transpose(x, (0, 2, 1)).astype(np.float16)  # [B, D, S]
    com = {
        "wq": np.asarray(Wq, np.float16),
        "wk": np.asarray(Wk, np.float16),
        "wv": np.asarray(Wv, np.float16),
        "wo": np.asarray(Wo, np.float16),
        "bq": np.asarray(bq, np.float32),
        "bk": np.asarray(bk, np.float32),
        "bv": np.asarray(bv, np.float32),
        "bo": np.asarray(bo, np.float32),
    }
    maps = []
    for c in range(8):
        b, h = divmod(c, 2)
        m = dict(com)
        # rotate so this core's query half occupies columns [0:NQ]
        m["xT"] = np.ascontiguousarray(np.roll(xT[b], -h * NQ, axis=1))
        maps.append(m)
    return maps


def kernel(x, Wq, bq, Wk, bk, Wv, bv, Wo, bo, _trace=False, _precise=None):
    nc = _get_nc()
    maps = _in_maps(x, Wq, bq, Wk, bk, Wv, bv, Wo, bo)
    res = run_bass_kernel_spmd(nc, maps, list(range(8)), trace=_trace)
    out = np.empty((B, S, D), np.float32)
    for c in range(8):
        b, h = divmod(c, 2)
        out[b, h * NQ : (h + 1) * NQ, :] = res.results[c]["zT"].T
    if _trace:
        kernel.last_exec_time_ns = res.exec_time_ns
        kernel.last_profile = res
    return out
